# revision 20
# baseline (speedup 1.0000x reference)
"""Contrastive pair loss on 8 Trainium2 NeuronCores.

loss = mean_b( relu(mean_i((z1[b,i]-z2[b,i])^2) - margin) )  for
z1, z2 of shape (1024, 256, 16, 16) fp32.

Sharding: data-parallel over the batch axis — each of the 8 cores gets 128
rows (one row = 65536 contiguous fp32, 32 MiB per tensor per core). The
host interleaves z1|z2 per tile into one array so each tile is a SINGLE
contiguous DMA (halves the DMA count and cross-engine waits). On-chip,
each core streams its shard through SBUF in [128, 2F] tiles: DVE computes
z1-z2 in place over the z2 half, ACT computes Square with a per-partition
accumulation (accum_out) into one acc slot, discarding its full-size
output through a stride-0 broadcast AP. All loads issue in order
on one ring and the tile widths taper (2048...256) so ACT is nearly
drained when the last bytes land and the post-last-byte serial chain is
short; the final 256-wide tile squares on DVE (mul+reduce), overlapping
ACT's last op. (Keep the body on ACT: DVE ops have a ~400 ns fixed cost,
so a DVE-side taper of many small ops serializes ~7 us;
tensor_tensor_reduce is rejected by this walrus build.) A DVE reduce collapses
the acc slots to per-row sums; a 32x32 block transpose packs the 128 row
sums onto 4 partitions x 32 lanes so the output DMA is 4 contiguous 128 B
lines instead of 128 scattered 4 B elements (the latter's 16-engine
completion trickle cost ~5 us). The hinge/mean epilogue over 1024 row
values runs on host.

Trace-driven structure choices:
- 4096-column body tiles = one 4 MiB 128-partition DMA each (32 KiB lines).
- All DMAs (loads and the lone output) issue in order from the SP HWDGE
  ring: completion order is predictable, and sync-ring 2D issues are
  ~585 ns vs ~1150 ns on the ACT ring for the strided output AP.
  (Splitting z1/z2 across both rings measured ~2 us worse.)
- The Tile end-of-kernel epilogue is patched lean: drain + gpsimd sem
  clear only (no EVSEM butterfly barriers) — the stock epilogue added
  ~290 EVENT_SEMAPHORE instructions and ~8 us.
"""

import numpy as np

B = 1024
CODE = 256 * 16 * 16  # 65536
N_CORES = 8
ROWS = B // N_CORES  # 128 rows per core == SBUF partition count
TILES = [4096] * 14 + [2048, 2048, 1536, 1024, 768, 512, 256]
NT = len(TILES)
N_ACT = NT - 1  # last (smallest) tile squares on DVE, overlapping ACT's tail
MARGIN = 0.01

_CACHE = {}


def _split_multi_waits(nc):
    """The walrus build in this image rejects instructions carrying more
    than one sync-wait command ("Too many sync wait commands",
    setupSyncWait). Tile routinely emits several waits on one instruction,
    so split them: for each instruction with N>1 waits, inject N-1
    single-wait NoOps on the same engine immediately before it. Same-engine
    program order makes this semantically identical."""
    from concourse import mybir

    k = 0
    for fn in nc.m.functions:
        for blk in fn.blocks:
            insts = blk.instructions
            out = []
            changed = False
            for ins in insts:
                si = ins.sync_info
                if si is not None and si.on_wait and len(si.on_wait) > 1:
                    waits = list(si.on_wait)
                    for w in waits[:-1]:
                        k += 1
                        nop = mybir.InstNoOp(
                            name=f"WSPLIT-{k}",
                            text_hint="split_wait",
                            bass_nofuse=True,
                        )
                        nop.engine = ins.engine
                        nop.sync_info = mybir.SyncInfo(on_wait=[w], on_update=[])
                        out.append(nop)
                    si.on_wait = waits[-1:]
                    ins.sync_info = si
                    changed = True
                out.append(ins)
            if changed:
                blk.instructions = out


def _patch_lean_epilogue():
    """Tile's kernel-tail epilogue is drain + EVSEM-butterfly barrier +
    sem clears + second butterfly. Replace it with: drain (carrying the
    global-clock sem waits) + the gpsimd dma_reset/sem_clear gated on the
    same waits. No engine barriers at all — waiting for every semaphore's
    final value is equivalent to waiting for all engines' last real work,
    and nothing executes after the clear."""
    from concourse.tile import TileContext, ScopedClock

    if getattr(TileContext, "_ant_lean_epilogue", False):
        return

    def _drain_and_barrier(self, tick_clock, wait_clock):
        nc = self.nc
        clock = ScopedClock({None: tick_clock.global_clock})
        drain_inst = nc.sync.drain()
        wait_clock.add_sem_waits(drain_inst.ins, clock)

        assert self.sems is not None
        popped = nc._tile_sem_poison_stack.pop()
        assert popped is self._sem_poison
        sems = list(self.sems.allocated().values())
        if sems:
            from concourse.bass import compact_to_ranges

            sem_nums = [s.num if hasattr(s, "num") else s for s in sems]
            first = True
            for sem_range in compact_to_ranges(sem_nums):
                assert nc._state.free_isdisjoint(sem_range)
                reset_inst = nc.gpsimd.dma_reset(sem_range)
                if first:
                    # gate the gpsimd-side clear on every sem reaching its
                    # final value, same condition as the drain
                    wait_clock.add_sem_waits(reset_inst.ins, clock)
                    first = False
                nc.gpsimd.sem_clear(sem_range)
            nc._state.prepend_free_semaphores(sem_nums)
            for poison_set in nc._tile_sem_poison_stack:
                poison_set.update(sem_nums)

    TileContext._drain_and_barrier = _drain_and_barrier
    TileContext._ant_lean_epilogue = True


def _build():
    if "nc" in _CACHE:
        return _CACHE["nc"]

    import concourse.bass as bass
    from concourse import mybir
    from concourse.tile import TileContext

    _patch_lean_epilogue()

    nc = bass.Bass("TRN2", target_bir_lowering=False, num_devices=N_CORES)
    zz = nc.dram_tensor("zz", [ROWS, 2 * CODE], mybir.dt.float32, kind="ExternalInput")
    out = nc.dram_tensor("out", [4, 32], mybir.dt.float32, kind="ExternalOutput")

    with TileContext(nc) as tc:
        with (
            tc.tile_pool(name="zp", bufs=6) as pz,
            tc.tile_pool(name="st", bufs=1) as ps,
        ):
            acc = ps.tile([ROWS, 32], mybir.dt.float32)
            dummy = ps.tile([ROWS, 1], mybir.dt.float32)
            tile_w = 2 * max(TILES)
            col = 0
            for j, f in enumerate(TILES):
                # host interleaves z1|z2 per tile: one DMA brings both halves
                t = pz.tile([ROWS, tile_w], mybir.dt.float32)
                nc.sync.dma_start(out=t[:, : 2 * f], in_=zz[:, col : col + 2 * f])
                nc.vector.tensor_sub(
                    out=t[:, f : 2 * f], in0=t[:, :f], in1=t[:, f : 2 * f]
                )
                if j < N_ACT:
                    nc.scalar.activation(
                        out=dummy[:].broadcast_to((ROWS, f)),
                        in_=t[:, f : 2 * f],
                        func=mybir.ActivationFunctionType.Square,
                        accum_out=acc[:, j : j + 1],
                    )
                else:
                    nc.vector.tensor_mul(
                        out=t[:, f : 2 * f], in0=t[:, f : 2 * f], in1=t[:, f : 2 * f]
                    )
                    nc.vector.tensor_reduce(
                        out=acc[:, j : j + 1],
                        in_=t[:, f : 2 * f],
                        axis=mybir.AxisListType.X,
                        op=mybir.AluOpType.add,
                    )
                col += 2 * f
            rs = ps.tile([ROWS, 32], mybir.dt.float32)
            nc.vector.tensor_reduce(
                out=rs[:, 0:1],
                in_=acc[:, 0:NT],
                axis=mybir.AxisListType.X,
                op=mybir.AluOpType.add,
            )
            tr = ps.tile([ROWS, 32], mybir.dt.float32)
            nc.vector.transpose(out=tr[:, :], in_=rs[:, :])
            # rowsum[32*b + r] lives at tr[32*b, r]; ship 4 partitions x
            # 128 B contiguous lines
            nc.sync.dma_start(out=out[:, :], in_=tr[0:ROWS:32, :])

    _split_multi_waits(nc)

    _CACHE["nc"] = nc
    return nc


def _run(z1, z2, trace=False, trace_cores=None):
    from concourse.bass_utils import run_bass_kernel_spmd

    nc = _build()
    z1f = np.asarray(z1, dtype=np.float32).reshape(B, CODE)
    z2f = np.asarray(z2, dtype=np.float32).reshape(B, CODE)
    # interleave z1|z2 per tile so each tile is a single contiguous DMA
    zzf = np.empty((B, 2 * CODE), dtype=np.float32)
    col = 0
    for f in TILES:
        zzf[:, 2 * col : 2 * col + f] = z1f[:, col : col + f]
        zzf[:, 2 * col + f : 2 * col + 2 * f] = z2f[:, col : col + f]
        col += f
    in_maps = [{"zz": zzf[c * ROWS : (c + 1) * ROWS]} for c in range(N_CORES)]
    res = run_bass_kernel_spmd(
        nc, in_maps, core_ids=list(range(N_CORES)), trace=trace,
        **({"trace_cores": trace_cores} if trace_cores else {}),
    )
    rowsum = np.concatenate(
        [res.results[c]["out"].reshape(-1) for c in range(N_CORES)]
    ).astype(np.float64)
    hamm = rowsum / CODE
    hinged = np.where(hamm > MARGIN, hamm - MARGIN, 0.0)
    loss = np.float32(hinged.sum() / B)
    return np.asarray(loss, dtype=np.float32), res


def kernel(z1, z2):
    return _run(z1, z2, trace=False)[0]
